# revision 1
# baseline (speedup 1.0000x reference)
"""Trainium2 Bass kernel for nn_BetaVAEMark7Decoder.

Strategy: all six layers are (blocks of) matmuls on the TensorEngine in
float32r. The up-conv/tconv pairs are fused on the host into composite
banded weight blocks (up1*tc1, up2*tc2, up3*tc3), so the device work is
pure data-parallel over batch: 4096 rows split 512 per NeuronCore.

Layouts: inner layers keep activations as [feature_partitions, (h, b)]
with the matmul moving dim = batch; the final fused layer swaps operands
(activations become the stationary lhsT, weights the moving rhs) so PSUM
comes out batch-major ([128 b, out-features]) and the NCHW output DMA is
contiguous per partition.
"""
import numpy as np
from contextlib import ExitStack

import concourse.bass as bass
import concourse.tile as tile
from concourse import bacc, mybir
from concourse.bass_utils import run_bass_kernel_spmd

F32 = mybir.dt.float32
F32R = mybir.dt.float32r
BF16 = mybir.dt.bfloat16
AF = mybir.ActivationFunctionType
OP = mybir.AluOpType

NCORES = 8
BCORE = 512          # batch rows per core
HALF = 512           # fused2/fused3 batch span (full)
CH = 128             # fused3 output-chunk batch size

X1_WIN = [(0, 3), (1, 4), (3, 4), (5, 3)]
X3_WIN = [(0, 5), (2, 7), (6, 7), (10, 6)]
A3_WIN = [(0, 12, 0, 10), (8, 12, 11, 18), (16, 12, 19, 26), (20, 12, 27, 31)]
A3_OWN = [(0, 0, 12), (1, 12, 20), (2, 20, 28), (3, 28, 32)]
HG = [(0, 2), (2, 2), (4, 1)]


# ---------------- host-side weight factorization ----------------
def _precompute(w):
    P = {}
    w_lin, b_lin = w["w_lin"], w["b_lin"]
    lhs_lin = np.zeros((7, 256), np.float32)
    c_lin = np.zeros(256, np.float32)
    for wi in range(8):
        for ci in range(32):
            lhs_lin[:, wi * 32 + ci] = w_lin[:, ci * 8 + wi]
            c_lin[wi * 32 + ci] = b_lin[ci * 8 + wi]
    P["lhs_lin"], P["c_lin"] = lhs_lin, c_lin

    w_up1, b_up1, w_tc1, b_tc1 = w["w_up1"], w["b_up1"], w["w_tc1"], w["b_tc1"]
    K1 = np.zeros((5, 2, 3, 32, 16), np.float32)
    for hh in range(5):
        for s in range(2):
            for dh in range(3):
                hp = hh + 1 - dh
                if not (0 <= hp < 5):
                    continue
                for dw in range(3):
                    t = s + 1 - dw
                    dj = int(np.floor(t / 2))
                    kw = t - 2 * dj
                    K1[hh, s, dj + 1] += np.einsum("ic,cd->id", w_up1[hp, kw], w_tc1[dh, dw])
    c1 = np.zeros((5, 16, 16), np.float32)
    for hh in range(5):
        for ww in range(16):
            acc = b_tc1.copy()
            for dh in range(3):
                if not (0 <= hh + 1 - dh < 5):
                    continue
                for dw in range(3):
                    if not (0 <= ww + 1 - dw < 16):
                        continue
                    acc = acc + b_up1 @ w_tc1[dh, dw]
            c1[hh, ww] = acc
    P["K1"], P["c1"] = K1, c1

    w_up2, b_up2, w_tc2, b_tc2 = w["w_up2"], w["b_up2"], w["w_tc2"], w["b_tc2"]
    K2 = np.zeros((5, 2, 3, 3, 16, 8), np.float32)
    for r in range(5):
        for s in range(2):
            for dh in range(3):
                u = r + 1 - dh
                di = int(np.floor(u / 5))
                kh = u - 5 * di
                for dw in range(3):
                    t = s + 1 - dw
                    dj = int(np.floor(t / 2))
                    kw = t - 2 * dj
                    K2[r, s, di + 1, dj + 1] += np.einsum("ic,cd->id", w_up2[kh, kw], w_tc2[dh, dw])
    P["K2"] = K2
    P["BB2"] = np.einsum("c,hwcd->hwd", b_up2, w_tc2)
    P["b_tc2"] = b_tc2

    w_up3, b_up3, w_tc3, b_tc3 = w["w_up3"], w["b_up3"], w["w_tc3"], w["b_tc3"]
    K3 = np.zeros((2, 2, 3, 3, 8, 6), np.float32)
    for r in range(2):
        for s in range(2):
            for dh in range(3):
                u = r + 1 - dh
                di = int(np.floor(u / 2))
                kh = u - 2 * di
                for dw in range(3):
                    t = s + 1 - dw
                    dj = int(np.floor(t / 2))
                    kw = t - 2 * dj
                    K3[r, s, di + 1, dj + 1] += np.einsum("ic,cd->id", w_up3[kh, kw], w_tc3[dh, dw])
    P["K3"] = K3
    P["BB3"] = np.einsum("c,hwcd->hwd", b_up3, w_tc3)
    P["b_tc3"] = b_tc3
    return P


def _fused1_blocks(P):
    K1 = P["K1"]
    blocks, biases = {}, {}
    for g, (h0, nh) in enumerate(HG):
        for a in range(4):
            wi0, nwi = X1_WIN[a]
            M = nh * 4 * 16
            B = np.zeros((nwi * 32, M), np.float32)
            bias = np.zeros(M, np.float32)
            for hi in range(nh):
                hh = h0 + hi
                for wl in range(4):
                    ww = 4 * a + wl
                    j, s = ww // 2, ww % 2
                    for c2 in range(16):
                        col = hi * 64 + wl * 16 + c2
                        bias[col] = P["c1"][hh, ww, c2]
                        for wi_l in range(nwi):
                            dj = (wi0 + wi_l) - j
                            if -1 <= dj <= 1:
                                B[wi_l * 32:(wi_l + 1) * 32, col] = K1[hh, s, dj + 1, :, c2]
            blocks[(g, a)] = B
            biases[(g, a)] = bias
    return blocks, biases


def _fused2_blocks(P):
    K2, BB2, b_tc2 = P["K2"], P["BB2"], P["b_tc2"]

    def col_bias(Hh, Ww, c3):
        acc = b_tc2[c3]
        for dh in range(3):
            if not (0 <= Hh + 1 - dh < 25):
                continue
            for dw in range(3):
                if not (0 <= Ww + 1 - dw < 32):
                    continue
                acc += BB2[dh, dw, c3]
        return acc

    blocks, biases = {}, {}
    for a in range(4):
        j0, nj = X3_WIN[a]
        Kr = nj * 16
        B = np.zeros((Kr, 128), np.float32)
        for ri, r in enumerate((1, 2)):
            for wl in range(8):
                Ww = 8 * a + wl
                j, s = Ww // 2, Ww % 2
                for c3 in range(8):
                    col = ri * 64 + wl * 8 + c3
                    for jl in range(nj):
                        dj = (j0 + jl) - j
                        if -1 <= dj <= 1:
                            B[jl * 16:(jl + 1) * 16, col] = K2[r, s, 1, dj + 1, :, c3]
        blocks[("r12", a)] = B
        bias = np.zeros(128, np.float32)
        for ri, r in enumerate((1, 2)):
            for wl in range(8):
                for c3 in range(8):
                    bias[ri * 64 + wl * 8 + c3] = col_bias(5 + r, 8 * a + wl, c3)
        biases[("r12", a)] = bias

        B = np.zeros((Kr, 64), np.float32)
        for wl in range(8):
            Ww = 8 * a + wl
            j, s = Ww // 2, Ww % 2
            for c3 in range(8):
                for jl in range(nj):
                    dj = (j0 + jl) - j
                    if -1 <= dj <= 1:
                        B[jl * 16:(jl + 1) * 16, wl * 8 + c3] = K2[3, s, 1, dj + 1, :, c3]
        blocks[("r3", a)] = B
        bias = np.zeros(64, np.float32)
        for wl in range(8):
            for c3 in range(8):
                bias[wl * 8 + c3] = col_bias(8, 8 * a + wl, c3)
        biases[("r3", a)] = bias

        for kind, r, dis in (("r0", 0, (-1, 0)), ("r4", 4, (0, 1))):
            for di in dis:
                B = np.zeros((Kr, 64), np.float32)
                for wl in range(8):
                    Ww = 8 * a + wl
                    j, s = Ww // 2, Ww % 2
                    for c3 in range(8):
                        for jl in range(nj):
                            dj = (j0 + jl) - j
                            if -1 <= dj <= 1:
                                B[jl * 16:(jl + 1) * 16, wl * 8 + c3] = K2[r, s, di + 1, dj + 1, :, c3]
                blocks[(kind, a, di)] = B
            # bias: interior-H version and edge version (i=0 for r0, i=4 for r4)
            for tag, i in (("mid", 2), ("edge", 0 if kind == "r0" else 4)):
                bias = np.zeros(64, np.float32)
                for wl in range(8):
                    for c3 in range(8):
                        bias[wl * 8 + c3] = col_bias(5 * i + r, 8 * a + wl, c3)
                biases[(kind, a, tag)] = bias
    return blocks, biases


def _fused3_blocks(P):
    """bf16 rhs blocks per (t, di[, iclass]): rows = (jl*8+ci) for jl<12, bias row 96.
    cols (r, s, jc, c4) r-major."""
    K3, BB3, b_tc3 = P["K3"], P["BB3"], P["b_tc3"]
    blocks = {}
    for t, (j0, nj, lo, hi) in enumerate(A3_WIN):
        njc = hi - lo + 1
        N = 4 * njc * 6

        def colidx(r, s, jc, c4):
            return ((r * 6 + c4) * njc + (jc - lo)) * 2 + s

        for di in (-1, 0, 1):
            B = np.zeros((97, N), np.float32)
            for r in range(2):
                for s in range(2):
                    for jc in range(lo, hi + 1):
                        for c4 in range(6):
                            col = colidx(r, s, jc, c4)
                            for jl in range(nj):
                                dj = (j0 + jl) - jc
                                if -1 <= dj <= 1:
                                    B[jl * 8:(jl + 1) * 8, col] = K3[r, s, di + 1, dj + 1, :, c4]
            if di != 0:
                blocks[(t, di)] = B
                continue
            for iclass in range(3):
                Bi = B.copy()
                for r in range(2):
                    for s in range(2):
                        for jc in range(lo, hi + 1):
                            for c4 in range(6):
                                acc = b_tc3[c4]
                                for dh in range(3):
                                    u = r + 1 - dh
                                    di_ = int(np.floor(u / 2))
                                    ok = (iclass == 0) or (iclass == 1 and di_ >= 0) \
                                        or (iclass == 2 and di_ <= 0)
                                    if not ok:
                                        continue
                                    for dw in range(3):
                                        tt = s + 1 - dw
                                        dj_ = int(np.floor(tt / 2))
                                        if 0 <= jc + dj_ < 32:
                                            acc += BB3[dh, dw, c4]
                                Bi[96, colidx(r, s, jc, c4)] = acc
                blocks[(t, 0, iclass)] = Bi
    return blocks


class _Pack:
    """Packs [K, M] blocks into one [128, cols] array; remembers offsets."""

    def __init__(self):
        self.cols = 0
        self.reg = {}
        self.items = []

    def add(self, key, arr):
        K, M = arr.shape
        self.reg[key] = (self.cols, K, M)
        self.items.append(arr)
        self.cols += M

    def build(self):
        out = np.zeros((128, self.cols), np.float32)
        c = 0
        for arr in self.items:
            K, M = arr.shape
            out[:K, c:c + M] = arr
            c += M
        return out


def _make_packs(inputs):
    P = _precompute(inputs)
    f1b, f1bias = _fused1_blocks(P)
    f2b, f2bias = _fused2_blocks(P)
    f3b = _fused3_blocks(P)

    wp = _Pack()
    wp.add("lin0", P["lhs_lin"][:, 0:128])
    wp.add("lin1", P["lhs_lin"][:, 128:256])
    for g in range(3):
        for a in range(4):
            wp.add(("f1", g, a), f1b[(g, a)])
    wb = _Pack()
    for a in range(4):
        wb.add(("f2r12", a), f2b[("r12", a)])
        wb.add(("f2r3", a), f2b[("r3", a)])
        for di in (-1, 0):
            wb.add(("f2r0", a, di), f2b[("r0", a, di)])
        for di in (0, 1):
            wb.add(("f2r4", a, di), f2b[("r4", a, di)])
    for t in range(4):
        for key in [(t, 0, 0), (t, 0, 1), (t, 0, 2), (t, -1), (t, 1)]:
            wb.add(("f3",) + key, f3b[key])

    bp = _Pack()
    bp.add("blin0", P["c_lin"][0:128].reshape(-1, 1))
    bp.add("blin1", P["c_lin"][128:256].reshape(-1, 1))
    for g in range(3):
        for a in range(4):
            bp.add(("b1", g, a), f1bias[(g, a)].reshape(-1, 1))
    for a in range(4):
        bp.add(("b2r12", a), f2bias[("r12", a)].reshape(-1, 1))
        bp.add(("b2r3", a), f2bias[("r3", a)].reshape(-1, 1))
        for tag in ("mid", "edge"):
            bp.add(("b2r0", a, tag), f2bias[("r0", a, tag)].reshape(-1, 1))
            bp.add(("b2r4", a, tag), f2bias[("r4", a, tag)].reshape(-1, 1))
    return wp, bp, wb


# ---------------- device program ----------------
_PROG = {}


def _lim(s):
    if s == 0:
        return 128
    if s == 64:
        return 64
    return 32


def _pieces(p0, d0, n):
    """Split a partition-range copy into HW-legal (offset, count) pieces.
    Both starts must be 32-aligned and each piece must obey the buddy rule."""
    assert p0 % 32 == 0 and d0 % 32 == 0, (p0, d0, n)
    out = []
    off = 0
    while off < n:
        s1, s2 = (p0 + off) % 128, (d0 + off) % 128
        c = min(n - off, _lim(s1), _lim(s2))
        out.append((off, c))
        off += c
    return out


def _build_program(wcols, bcols, wbcols):
    key = (wcols, bcols, wbcols)
    if key in _PROG:
        return _PROG[key]
    nc = bacc.Bacc("TRN2", target_bir_lowering=False, debug=False, num_devices=NCORES)
    lat_ap = nc.dram_tensor("latent", [BCORE, 7], F32, kind="ExternalInput").ap()
    wp_ap = nc.dram_tensor("wpack", [128, wcols], F32, kind="ExternalInput").ap()
    bp_ap = nc.dram_tensor("bpack", [128, bcols], F32, kind="ExternalInput").ap()
    wb_ap = nc.dram_tensor("wbpack", [128, wbcols], BF16, kind="ExternalInput").ap()
    out_ap = nc.dram_tensor("out", [BCORE, 6, 50, 64], F32, kind="ExternalOutput").ap()
    with tile.TileContext(nc) as tc:
        with ExitStack() as ctx:
            _emit(ctx, tc, nc, lat_ap, wp_ap, bp_ap, wb_ap, out_ap,
                  _build_program.wreg, _build_program.breg, _build_program.wbreg)
    nc.compile()
    _PROG[key] = nc
    return nc


def _emit(ctx, tc, nc, lat_ap, wp_ap, bp_ap, wb_ap, out_ap, wreg, breg, wbreg):
    wcols = wp_ap.shape[1]
    bcols = bp_ap.shape[1]
    wbcols = wb_ap.shape[1]

    consts = ctx.enter_context(tc.tile_pool(name="consts", bufs=1))
    bounce = ctx.enter_context(tc.tile_pool(name="bounce", bufs=2))
    x1p = ctx.enter_context(tc.tile_pool(name="x1", bufs=1))
    x3p = ctx.enter_context(tc.tile_pool(name="x3", bufs=1))
    a3p = ctx.enter_context(tc.tile_pool(name="a3", bufs=1))
    stgp = ctx.enter_context(tc.tile_pool(name="stg", bufs=4))
    tmpp = ctx.enter_context(tc.tile_pool(name="tmp", bufs=4))
    ps_ctx = ExitStack()
    psmall = ps_ctx.enter_context(tc.tile_pool(name="psA", bufs=2, space="PSUM"))

    # ---- constants ----
    wp_r = consts.tile([128, wcols], F32R)
    for c0 in range(0, wcols, 512):
        n = min(512, wcols - c0)
        bt = bounce.tile([128, 512], F32, tag="bounce", name=f"bw{c0}")
        nc.sync.dma_start(bt[:, :n], wp_ap[:, c0:c0 + n])
        nc.vector.tensor_copy(wp_r[:, c0:c0 + n], bt[:, :n])
    wbt = consts.tile([128, wbcols], BF16)
    nc.sync.dma_start(wbt[:], wb_ap[:])
    bpt = consts.tile([128, bcols], F32)
    nc.sync.dma_start(bpt[:], bp_ap[:])
    lat_f = consts.tile([7, BCORE], F32)
    nc.sync.dma_start(lat_f[:], lat_ap[:].rearrange("b d -> d b"))
    lat_r = consts.tile([7, BCORE], F32R)
    nc.vector.tensor_copy(lat_r[:], lat_f[:])

    def W(key):
        o, K, M = wreg[key]
        return wp_r[:K, o:o + M]

    def WB(key):
        o, K, M = wbreg[key]
        return wbt[:K, o:o + M]

    def BV(key, p0, n):
        o, K, M = breg[key]
        return bpt[p0:p0 + n, o:o + 1]

    def act_lrelu(dst, src, bias):
        nc.scalar.activation(dst, src, AF.Lrelu, bias=bias, scale=1.0, alpha=0.01)

    def evac(dst_tile, d0, ps, p0, n, bkey, fsl_out, fsl_in=None):
        """lrelu+bias evacuation with partition legality splitting."""
        fsl_in = fsl_in if fsl_in is not None else slice(None)
        for off, cnt in _pieces(p0, d0, n):
            act_lrelu(dst_tile[d0 + off:d0 + off + cnt, fsl_out],
                      ps[p0 + off:p0 + off + cnt, fsl_in],
                      BV(bkey, p0 + off, cnt))

    # ---- lin ----
    psA = psmall.tile([128, BCORE], F32, tag="ps")
    nc.tensor.matmul(psA[:], W("lin0"), lat_r[:], start=True, stop=True)
    psB = psmall.tile([128, BCORE], F32, tag="ps")
    nc.tensor.matmul(psB[:], W("lin1"), lat_r[:], start=True, stop=True)

    x1t = [x1p.tile([X1_WIN[a][1] * 32, BCORE], F32R, tag=f"x1_{a}", name=f"x1_{a}")
           for a in range(4)]
    # x1 window a holds wi in [wi0, wi0+nwi); fill from psA (wi 0..3) / psB (4..7)
    for a in range(4):
        wi0, nwi = X1_WIN[a]
        for ps, base, bkey in ((psA, 0, "blin0"), (psB, 4, "blin1")):
            lo = max(wi0, base)
            hi = min(wi0 + nwi, base + 4)
            if lo >= hi:
                continue
            evac(x1t[a], (lo - wi0) * 32, ps, (lo - base) * 32, (hi - lo) * 32, bkey,
                 slice(None))

    # ---- fused1 -> x3 windows ----
    x3t = [x3p.tile([X3_WIN[a][1] * 16, 5 * BCORE], BF16, tag=f"x3_{a}", name=f"x3_{a}")
           for a in range(4)]
    for g, (h0, nh) in enumerate(HG):
        for a in range(4):
            M = nh * 64
            ps = psmall.tile([M, BCORE], F32, tag="ps")
            nc.tensor.matmul(ps[:], W(("f1", g, a)), x1t[a][:], start=True, stop=True)
            for hi_ in range(nh):
                hh = h0 + hi_
                for b_ in range(4):
                    jb0, njb = X3_WIN[b_]
                    w_lo = max(4 * a, jb0)
                    w_hi = min(4 * a + 4, jb0 + njb)
                    if w_lo >= w_hi:
                        continue
                    evac(x3t[b_], (w_lo - jb0) * 16,
                         ps, hi_ * 64 + (w_lo - 4 * a) * 16, (w_hi - w_lo) * 16,
                         ("b1", g, a),
                         slice(hh * BCORE, (hh + 1) * BCORE))

    # ---- fused2 + fused3 per half ----
    a3t = []
    for t, (j0, nj, lo, hi) in enumerate(A3_WIN):
        at = a3p.tile([97, 25 * HALF], BF16, tag=f"a3_{t}", name=f"a3_{t}")
        a3t.append(at)
        nc.gpsimd.memset(at[96:97, :], 1.0)

    def a3_targets(w_lo, w_hi):
        res = []
        for t, o_lo, o_hi in A3_OWN:
            lo_ = max(w_lo, o_lo)
            hi_ = min(w_hi, o_hi)
            if lo_ < hi_:
                res.append((t, lo_, hi_))
        return res

    f2ev = [0]

    def f2_evac(ps, p0, Hh, w_lo, w_hi, bkey, hb):
        for t, lo_, hi_ in a3_targets(w_lo, w_hi):
            d0 = (lo_ - A3_WIN[t][0]) * 8
            pr0 = p0 + (lo_ - w_lo) * 8
            n = (hi_ - lo_) * 8
            fsl = slice(Hh * HALF, (Hh + 1) * HALF)
            for off, cnt in _pieces(pr0, d0, n):
                act_lrelu(a3t[t][d0 + off:d0 + off + cnt, fsl],
                          ps[pr0 + off:pr0 + off + cnt, :],
                          BV(bkey, pr0 + off, cnt))

    for half in range(1):
        hb = 0

        def xsl(i):
            return slice(i * BCORE + hb, i * BCORE + hb + HALF)

        for i in range(5):
            for a in range(4):
                ps = psmall.tile([128, HALF], F32, tag="ps", name=f"p12_{half}_{i}_{a}")
                nc.tensor.matmul(ps[:], WB(("f2r12", a)), x3t[a][:, xsl(i)],
                                 start=True, stop=True)
                for ri, r in enumerate((1, 2)):
                    f2_evac(ps, ri * 64, 5 * i + r, 8 * a, 8 * a + 8,
                            ("b2r12", a), hb)
                ps = psmall.tile([64, HALF], F32, tag="ps", name=f"p3_{half}_{i}_{a}")
                nc.tensor.matmul(ps[:], WB(("f2r3", a)), x3t[a][:, xsl(i)],
                                 start=True, stop=True)
                f2_evac(ps, 0, 5 * i + 3, 8 * a, 8 * a + 8, ("b2r3", a), hb)
                ps = psmall.tile([64, HALF], F32, tag="ps", name=f"p0_{half}_{i}_{a}")
                nc.tensor.matmul(ps[:], WB(("f2r0", a, 0)), x3t[a][:, xsl(i)],
                                 start=True, stop=(i == 0))
                if i > 0:
                    nc.tensor.matmul(ps[:], WB(("f2r0", a, -1)), x3t[a][:, xsl(i - 1)],
                                     start=False, stop=True)
                f2_evac(ps, 0, 5 * i, 8 * a, 8 * a + 8,
                        ("b2r0", a, "edge" if i == 0 else "mid"), hb)
                ps = psmall.tile([64, HALF], F32, tag="ps", name=f"p4_{half}_{i}_{a}")
                nc.tensor.matmul(ps[:], WB(("f2r4", a, 0)), x3t[a][:, xsl(i)],
                                 start=True, stop=(i == 4))
                if i < 4:
                    nc.tensor.matmul(ps[:], WB(("f2r4", a, 1)), x3t[a][:, xsl(i + 1)],
                                     start=False, stop=True)
                f2_evac(ps, 0, 5 * i + 4, 8 * a, 8 * a + 8,
                        ("b2r4", a, "edge" if i == 4 else "mid"), hb)

        # halo mirrors: t1 j 8..11 <- t0 rows 64..96; t2 j 16..19 <- t1 rows 64..96;
        # t3 j 20..27 <- t2 rows 32..96
        for dst, src_t, s0, d0, n in ((1, 0, 64, 0, 32), (2, 1, 64, 0, 32),
                                      (3, 2, 32, 0, 64)):
            for i5 in range(5):
                fsl = slice(i5 * 5 * HALF, (i5 + 1) * 5 * HALF)
                nc.sync.dma_start(a3t[dst][d0:d0 + n, fsl],
                                  a3t[src_t][s0:s0 + n, fsl])
        ps_ctx.close()
        pf3 = ctx.enter_context(tc.tile_pool(name="psB", bufs=4, space="PSUM"))

        # ---- fused3 ----
        stg_cnt = [0]
        for c in range(4):
            cb = c * CH
            for ip0 in range(0, 25, 2):
                np_ = min(2, 25 - ip0)
                stg = stgp.tile([128, 768 * np_], F32, tag="stg",
                                name=f"stg_{half}_{c}_{ip0}")
                stv = stg[:].rearrange("p (c4 hq jc s) -> p hq c4 jc s",
                                       c4=6, hq=2 * np_, jc=32, s=2)
                for ii in range(np_):
                    i = ip0 + ii
                    iclass = 1 if i == 0 else (2 if i == 24 else 0)
                    for t, (j0, nj, lo, hi) in enumerate(A3_WIN):
                        njc = hi - lo + 1
                        N = 4 * njc * 6
                        hN = N // 2

                        def lsl(ix):
                            return a3t[t][:, ix * HALF + cb: ix * HALF + cb + CH]

                        ps3 = pf3.tile([128, 264], F32, tag=f"f3_{t % 2}",
                                       name=f"ps3_{half}_{c}_{i}_{t}")
                        ps3 = ps3[:, 0:N]
                        mms = [(ps3[:, 0:N], WB(("f3", t, 0, iclass)), lsl(i))]
                        if i > 0:
                            mms.append((ps3[:, 0:hN], WB(("f3", t, -1))[:, 0:hN],
                                        lsl(i - 1)))
                        if i < 24:
                            mms.append((ps3[:, hN:N], WB(("f3", t, 1))[:, hN:N],
                                        lsl(i + 1)))
                        for k, (o_, w_, l_) in enumerate(mms):
                            nc.tensor.matmul(o_, l_, w_, start=(k == 0),
                                             stop=(k == len(mms) - 1),
                                             skip_group_check=True)
                        view = stv[:, 2 * ii:2 * ii + 2, :, lo:hi + 1, :]
                        k13 = stg_cnt[0] % 10
                        stg_cnt[0] += 1
                        if k13 < 5:
                            tmp = tmpp.tile([128, 264], F32, tag="f3tmp",
                                            name=f"tmp_{half}_{c}_{i}_{t}")
                            nc.vector.tensor_copy(tmp[:, :N], ps3[:])
                            for r_ in range(2):
                                vr = stv[:, 2 * ii + r_:2 * ii + r_ + 1, :,
                                         lo:hi + 1, :].squeeze(1)
                                nc.vector.scalar_tensor_tensor(
                                    vr, tmp[:, r_ * hN:(r_ + 1) * hN],
                                    0.01, tmp[:, r_ * hN:(r_ + 1) * hN],
                                    op0=OP.mult, op1=OP.max)
                        else:
                            nc.scalar.activation(view, ps3[:], AF.Lrelu, bias=0.0,
                                                 scale=1.0, alpha=0.01)
                bg = hb + cb
                dview = out_ap[bg:bg + CH, :, 2 * ip0:2 * ip0 + 2 * np_, :]
                sview = stg[:].rearrange("p (c h w) -> p c h w",
                                         c=6, h=2 * np_, w=64)
                nc.sync.dma_start(dview, sview)


def kernel(**inputs):
    inputs = {k: np.asarray(v) for k, v in inputs.items()}
    wp, bp, wb = _make_packs(inputs)
    wpack = wp.build()
    bpack = bp.build()
    import ml_dtypes
    wbpack = wb.build().astype(ml_dtypes.bfloat16)
    _build_program.wreg = wp.reg
    _build_program.breg = bp.reg
    _build_program.wbreg = wb.reg
    nc = _build_program(wpack.shape[1], bpack.shape[1], wbpack.shape[1])

    lat = np.ascontiguousarray(inputs["latent"].astype(np.float32))
    in_maps = [
        {"latent": lat[i * BCORE:(i + 1) * BCORE], "wpack": wpack,
         "bpack": bpack, "wbpack": wbpack}
        for i in range(NCORES)
    ]
    res = run_bass_kernel_spmd(nc, in_maps, core_ids=list(range(NCORES)))
    return np.concatenate([res.results[i]["out"] for i in range(NCORES)], axis=0)



# revision 5
# speedup vs baseline: 1.6645x; 1.6645x over previous
"""Trainium2 Bass kernel for nn_BetaVAEMark7Decoder (v2).

All six layers are banded matmuls on the TensorEngine in bf16, data-parallel
over batch (4096 rows -> 512 per NeuronCore).  Biases ride as extra rows of
the stationary operands (activation tiles carry a constant ones-row), so every
PSUM evacuation is a single bias-free leaky-relu instruction, alternated
between the Scalar(ACT) and Vector(DVE) engines.  The final layer is blocked
on odd output-row boundaries (slot k = rows {2k+1,2k+2}) so each input slice
feeds exactly two PSUM slots and each stationary serves two full-width
matmuls.  Output is staged batch-major in bf16 and written with 8 large
contiguous DMAs; the host upcasts to float32.
"""
import numpy as np
from contextlib import ExitStack

import concourse.bass as bass
import concourse.tile as tile
from concourse import bacc, mybir
from concourse.bass_utils import run_bass_kernel_spmd

F32 = mybir.dt.float32
BF16 = mybir.dt.bfloat16
AF = mybir.ActivationFunctionType
OP = mybir.AluOpType

NCORES = 8
BCORE = 512
CH = 128

# x3 (= post-fused1 activation, j in 0..15, c2 in 0..15) windows: (j0, nj)
X3_WIN = [(0, 5), (2, 7), (6, 7), (10, 6)]
# x3 fill ownership (j ranges, even boundaries) per window
X3_OWN = [(0, 2), (2, 8), (8, 12), (12, 16)]
# x1 V-windows: V_k holds wi in {k, k+1, k+2}, ones row at 96
NV = 6
# a3 (= x4 activation, W in 0..31, c3 in 0..7) windows for fused3: (W0, nW)
A3_WIN = [(0, 15), (12, 15), (20, 12)]
# a3 fill ownership (W ranges) per window
A3_OWN = [(0, 12), (12, 20), (20, 32)]
# fused3 weight-col ownership (jc ranges) per window
A3_JC = [(0, 13), (13, 26), (26, 32)]


# ---------------- host-side weight factorization ----------------
def _precompute(w):
    P = {}
    w_lin, b_lin = w["w_lin"], w["b_lin"]
    lhs_lin = np.zeros((7, 256), np.float32)
    c_lin = np.zeros(256, np.float32)
    for wi in range(8):
        for ci in range(32):
            lhs_lin[:, wi * 32 + ci] = w_lin[:, ci * 8 + wi]
            c_lin[wi * 32 + ci] = b_lin[ci * 8 + wi]
    P["lhs_lin"], P["c_lin"] = lhs_lin, c_lin

    w_up1, b_up1, w_tc1, b_tc1 = w["w_up1"], w["b_up1"], w["w_tc1"], w["b_tc1"]
    K1 = np.zeros((5, 2, 3, 32, 16), np.float32)
    for hh in range(5):
        for s in range(2):
            for dh in range(3):
                hp = hh + 1 - dh
                if not (0 <= hp < 5):
                    continue
                for dw in range(3):
                    t = s + 1 - dw
                    dj = int(np.floor(t / 2))
                    kw = t - 2 * dj
                    K1[hh, s, dj + 1] += np.einsum("ic,cd->id", w_up1[hp, kw], w_tc1[dh, dw])
    c1 = np.zeros((5, 16, 16), np.float32)
    for hh in range(5):
        for ww in range(16):
            acc = b_tc1.copy()
            for dh in range(3):
                if not (0 <= hh + 1 - dh < 5):
                    continue
                for dw in range(3):
                    if not (0 <= ww + 1 - dw < 16):
                        continue
                    acc = acc + b_up1 @ w_tc1[dh, dw]
            c1[hh, ww] = acc
    P["K1"], P["c1"] = K1, c1

    w_up2, b_up2, w_tc2, b_tc2 = w["w_up2"], w["b_up2"], w["w_tc2"], w["b_tc2"]
    K2 = np.zeros((5, 2, 3, 3, 16, 8), np.float32)
    for r in range(5):
        for s in range(2):
            for dh in range(3):
                u = r + 1 - dh
                di = int(np.floor(u / 5))
                kh = u - 5 * di
                for dw in range(3):
                    t = s + 1 - dw
                    dj = int(np.floor(t / 2))
                    kw = t - 2 * dj
                    K2[r, s, di + 1, dj + 1] += np.einsum("ic,cd->id", w_up2[kh, kw], w_tc2[dh, dw])
    P["K2"] = K2
    P["BB2"] = np.einsum("c,hwcd->hwd", b_up2, w_tc2)
    P["b_tc2"] = b_tc2

    w_up3, b_up3, w_tc3, b_tc3 = w["w_up3"], w["b_up3"], w["w_tc3"], w["b_tc3"]
    K3 = np.zeros((2, 2, 3, 3, 8, 6), np.float32)
    for r in range(2):
        for s in range(2):
            for dh in range(3):
                u = r + 1 - dh
                di = int(np.floor(u / 2))
                kh = u - 2 * di
                for dw in range(3):
                    t = s + 1 - dw
                    dj = int(np.floor(t / 2))
                    kw = t - 2 * dj
                    K3[r, s, di + 1, dj + 1] += np.einsum("ic,cd->id", w_up3[kh, kw], w_tc3[dh, dw])
    P["K3"] = K3
    P["BB3"] = np.einsum("c,hwcd->hwd", b_up3, w_tc3)
    P["b_tc3"] = b_tc3
    return P


def _fused1_blocks(P):
    """Per a' (j-pair {2a',2a'+1}): A block [97, 128] (H rows 0..3) and
    B block [97, 32] (H row 4).  Rows = V-window wi*32+ci, bias at row 96."""
    K1, c1 = P["K1"], P["c1"]
    blocks = {}
    for ap_ in range(8):
        k = min(max(ap_ - 1, 0), NV - 1)  # V-window index
        A = np.zeros((97, 128), np.float32)
        B = np.zeros((97, 32), np.float32)
        for wl in range(2):
            j = 2 * ap_ + wl
            ju, s = j // 2, j % 2
            for hh in range(5):
                for c2 in range(16):
                    if hh < 4:
                        col = hh * 32 + wl * 16 + c2
                        dst = A
                    else:
                        col = wl * 16 + c2
                        dst = B
                    dst[96, col] = c1[hh, j, c2]
                    for wi_l in range(3):
                        wi = k + wi_l
                        if wi > 7:
                            continue
                        dj = wi - ju
                        if -1 <= dj <= 1:
                            dst[wi_l * 32:(wi_l + 1) * 32, col] = K1[hh, s, dj + 1, :, c2]
        blocks[("f1a", ap_)] = A
        blocks[("f1b", ap_)] = B
    return blocks


def _fused2_blocks(P):
    """Baseline banded blocks with bias rows appended (row K = bias).
    Keys: (kind, a[, variant]).  Halo blocks have zero bias rows."""
    K2, BB2, b_tc2 = P["K2"], P["BB2"], P["b_tc2"]

    def col_bias(Hh, Ww, c3):
        acc = b_tc2[c3]
        for dh in range(3):
            if not (0 <= Hh + 1 - dh < 25):
                continue
            for dw in range(3):
                if not (0 <= Ww + 1 - dw < 32):
                    continue
                acc += BB2[dh, dw, c3]
        return acc

    blocks = {}
    for a in range(4):
        j0, nj = X3_WIN[a]
        Kr = nj * 16
        B = np.zeros((Kr + 1, 128), np.float32)
        for ri, r in enumerate((1, 2)):
            for wl in range(8):
                Ww = 8 * a + wl
                j, s = Ww // 2, Ww % 2
                for c3 in range(8):
                    col = ri * 64 + wl * 8 + c3
                    B[Kr, col] = col_bias(5 * 0 + r, Ww, c3)  # placeholder, fixed below
                    for jl in range(nj):
                        dj = (j0 + jl) - j
                        if -1 <= dj <= 1:
                            B[jl * 16:(jl + 1) * 16, col] = K2[r, s, 1, dj + 1, :, c3]
        # r=1,2 biases are H-interior for every i (rows 5i+1, 5i+2 never clip)
        for ri, r in enumerate((1, 2)):
            for wl in range(8):
                for c3 in range(8):
                    B[Kr, ri * 64 + wl * 8 + c3] = col_bias(5 + r, 8 * a + wl, c3)
        blocks[("r12", a)] = B

        B = np.zeros((Kr + 1, 64), np.float32)
        for wl in range(8):
            Ww = 8 * a + wl
            j, s = Ww // 2, Ww % 2
            for c3 in range(8):
                col = wl * 8 + c3
                B[Kr, col] = col_bias(8, Ww, c3)
                for jl in range(nj):
                    dj = (j0 + jl) - j
                    if -1 <= dj <= 1:
                        B[jl * 16:(jl + 1) * 16, col] = K2[3, s, 1, dj + 1, :, c3]
        blocks[("r3", a)] = B

        for kind, r, dis in (("r0", 0, (-1, 0)), ("r4", 4, (0, 1))):
            for di in dis:
                main = di == 0
                W = np.zeros((Kr + 1, 64), np.float32)
                for wl in range(8):
                    Ww = 8 * a + wl
                    j, s = Ww // 2, Ww % 2
                    for c3 in range(8):
                        for jl in range(nj):
                            dj = (j0 + jl) - j
                            if -1 <= dj <= 1:
                                W[jl * 16:(jl + 1) * 16, wl * 8 + c3] = K2[r, s, di + 1, dj + 1, :, c3]
                if not main:
                    blocks[(kind + "h", a)] = W
                    continue
                for tag, i in (("mid", 2), ("edge", 0 if kind == "r0" else 4)):
                    Wv = W.copy()
                    for wl in range(8):
                        for c3 in range(8):
                            Wv[Kr, wl * 8 + c3] = col_bias(5 * i + r, 8 * a + wl, c3)
                    blocks[(kind + "m", a, tag)] = Wv
    return blocks


def _fused3_blocks(P):
    """Slot-prime blocks.  Per window t: rows (W-W0)*8+c3, ones row at 8*nW.
    Cols ordered (q, c4, jc-own, s).  U: q=0 -> row 2i-1 (di=+1), q=1 -> row 2i
    (di=0, carries bias).  L: q=0 -> row 2i+1 (di=0, bias), q=1 -> row 2i+2
    (di=-1).  U0 = q=1 half with H'=0 edge bias; L24 = q=0 half, H'=49 edge."""
    K3, BB3, b_tc3 = P["K3"], P["BB3"], P["b_tc3"]

    def col_bias(jc, s, c4, drop_dh):
        acc = b_tc3[c4]
        for dh in range(3):
            if dh in drop_dh:
                continue
            for dw in range(3):
                tt = s + 1 - dw
                dj = int(np.floor(tt / 2))
                if 0 <= jc + dj < 32:
                    acc += BB3[dh, dw, c4]
        return acc

    blocks = {}
    for t in range(3):
        W0, nW = A3_WIN[t]
        lo, hi = A3_JC[t]
        njc = hi - lo
        K = 8 * nW  # ones row index
        half = 6 * njc * 2

        def fill(B, q, r, di, bias_drop=None):
            """bias_drop None -> no bias row; else set of dh to drop."""
            for c4 in range(6):
                for jci in range(njc):
                    jc = lo + jci
                    for s in range(2):
                        col = q * half + (c4 * njc + jci) * 2 + s
                        if bias_drop is not None:
                            B[K, col] = col_bias(jc, s, c4, bias_drop)
                        for dj in (-1, 0, 1):
                            j = jc + dj
                            if not (W0 <= j < W0 + nW) or not (0 <= j < 32):
                                continue
                            jl = j - W0
                            B[jl * 8:(jl + 1) * 8, col] = K3[r, s, di + 1, dj + 1, :, c4]

        U = np.zeros((K + 1, 2 * half), np.float32)
        fill(U, 0, 1, +1)                      # row 2i-1
        fill(U, 1, 0, 0, bias_drop=set())      # row 2i, interior bias
        blocks[("U", t)] = U
        L = np.zeros((K + 1, 2 * half), np.float32)
        fill(L, 0, 1, 0, bias_drop=set())      # row 2i+1, interior bias
        fill(L, 1, 0, -1)                      # row 2i+2
        blocks[("L", t)] = L
        U0 = np.zeros((K + 1, 2 * half), np.float32)
        fill(U0, 1, 0, 0, bias_drop={2})       # row 0 edge bias
        blocks[("U0", t)] = U0[:, half:]
        L24 = np.zeros((K + 1, 2 * half), np.float32)
        fill(L24, 0, 1, 0, bias_drop={0})      # row 49 edge bias
        blocks[("L24", t)] = L24[:, :half]
    return blocks


class _Pack:
    """Packs [K, M] blocks into one [128, cols] array; remembers offsets."""

    def __init__(self):
        self.cols = 0
        self.reg = {}
        self.items = []

    def add(self, key, arr):
        K, M = arr.shape
        self.reg[key] = (self.cols, K, M)
        self.items.append(arr)
        self.cols += M

    def build(self):
        out = np.zeros((128, self.cols), np.float32)
        c = 0
        for arr in self.items:
            K, M = arr.shape
            out[:K, c:c + M] = arr
            c += M
        return out


def _make_packs(inputs):
    P = _precompute(inputs)
    f1b = _fused1_blocks(P)
    f2b = _fused2_blocks(P)
    f3b = _fused3_blocks(P)

    wp = _Pack()
    lin0 = np.zeros((8, 128), np.float32)
    lin0[:7] = P["lhs_lin"][:, 0:128]
    lin0[7] = P["c_lin"][0:128]
    lin1 = np.zeros((8, 128), np.float32)
    lin1[:7] = P["lhs_lin"][:, 128:256]
    lin1[7] = P["c_lin"][128:256]
    wp.add("lin0", lin0)
    wp.add("lin1", lin1)
    for ap_ in range(8):
        wp.add(("f1a", ap_), f1b[("f1a", ap_)])
        wp.add(("f1b", ap_), f1b[("f1b", ap_)])
    for a in range(4):
        wp.add(("r12", a), f2b[("r12", a)])
        wp.add(("r3", a), f2b[("r3", a)])
        for tag in ("mid", "edge"):
            wp.add(("r0m", a, tag), f2b[("r0m", a, tag)])
            wp.add(("r4m", a, tag), f2b[("r4m", a, tag)])
        wp.add(("r0h", a), f2b[("r0h", a)])
        wp.add(("r4h", a), f2b[("r4h", a)])
    for t in range(3):
        for kind in ("U", "L", "U0", "L24"):
            wp.add((kind, t), f3b[(kind, t)])
    return wp


# ---------------- device program ----------------
_PROG = {}


def _lim(s):
    s = s % 128
    if s == 0:
        return 128
    if s == 64:
        return 64
    return 32


def _pieces(p0, d0, n):
    """Split a partition-range copy into HW-legal (offset, count) pieces."""
    assert p0 % 32 == 0 and d0 % 32 == 0, (p0, d0, n)
    out = []
    off = 0
    while off < n:
        s1, s2 = (p0 + off) % 128, (d0 + off) % 128
        c = min(n - off, _lim(s1), _lim(s2))
        out.append((off, c))
        off += c
    return out


def _build_program(wcols):
    key = (wcols,)
    if key in _PROG:
        return _PROG[key]
    nc = bacc.Bacc("TRN2", target_bir_lowering=False, debug=False, num_devices=NCORES)
    lat_ap = nc.dram_tensor("latT", [8, BCORE], BF16, kind="ExternalInput").ap()
    wp_ap = nc.dram_tensor("wpack", [128, wcols], BF16, kind="ExternalInput").ap()
    ones_ap = nc.dram_tensor("ones", [1, 12800], BF16, kind="ExternalInput").ap()
    out_ap = nc.dram_tensor("out", [BCORE, 6, 50, 64], BF16, kind="ExternalOutput").ap()
    with tile.TileContext(nc) as tc:
        with ExitStack() as ctx:
            _emit(ctx, tc, nc, lat_ap, wp_ap, ones_ap, out_ap, _build_program.wreg)
    nc.compile()
    _PROG[key] = nc
    return nc


def _emit(ctx, tc, nc, lat_ap, wp_ap, ones_ap, out_ap, wreg):
    wcols = wp_ap.shape[1]

    consts = ctx.enter_context(tc.tile_pool(name="consts", bufs=1))
    x1p = ctx.enter_context(tc.tile_pool(name="x1", bufs=1))
    x3p = ctx.enter_context(tc.tile_pool(name="x3", bufs=1))
    a3p = ctx.enter_context(tc.tile_pool(name="a3", bufs=1))
    stgp = ctx.enter_context(tc.tile_pool(name="stg", bufs=2))
    tmpp = ctx.enter_context(tc.tile_pool(name="tmp", bufs=4))
    ps_ctx = ExitStack()
    pp128 = ps_ctx.enter_context(tc.tile_pool(name="pp128", bufs=2, space="PSUM"))
    pp64 = ps_ctx.enter_context(tc.tile_pool(name="pp64", bufs=6, space="PSUM"))

    # ---- constants ----
    wbt = consts.tile([128, wcols], BF16)
    nc.sync.dma_start(wbt[:], wp_ap[:])
    latT = consts.tile([8, BCORE], BF16)
    nc.sync.dma_start(latT[:], lat_ap[:])

    def W(key):
        o, K, M = wreg[key]
        return wbt[:K, o:o + M]

    # activation tiles (+1 ones row each)
    x1t = [x1p.tile([97, BCORE], BF16, name=f"x1_{k}") for k in range(NV)]
    x3t = [x3p.tile([X3_WIN[a][1] * 16 + 1, 5 * BCORE], BF16, name=f"x3_{a}")
           for a in range(4)]
    a3K = [A3_WIN[t][1] * 8 for t in range(3)]
    a3t = [a3p.tile([a3K[t] + 1, 25 * BCORE], BF16, name=f"a3_{t}") for t in range(3)]
    for k in range(NV):
        nc.sync.dma_start(x1t[k][96:97, :], ones_ap[:, 0:BCORE])
    for a in range(4):
        K = X3_WIN[a][1] * 16
        nc.sync.dma_start(x3t[a][K:K + 1, :], ones_ap[:, 0:5 * BCORE])
    for t in range(3):
        nc.sync.dma_start(a3t[t][a3K[t]:a3K[t] + 1, :], ones_ap[:, 0:25 * BCORE])

    # ---- evac dispatcher: lrelu(psum) -> dst, alternating ACT / DVE ----
    ev_n = [0]

    def lrelu_evac(dst, ps, np_, nf, dve_subs=None):
        """dve_subs: optional [(dst_i, c0, c1)] column split when dst is >3D
        (the DVE TensorScalarPtr out AP is limited to 3 canonical dims)."""
        k = ev_n[0]
        ev_n[0] += 1
        if k % 3 < 2:
            nc.scalar.activation(dst, ps, AF.Lrelu, bias=0.0, scale=1.0, alpha=0.01)
        else:
            tmp = tmpp.tile([128, 512], BF16, tag="evt", name=f"evt{k}")
            tv = tmp[0:np_, 0:nf]
            nc.vector.tensor_copy(tv, ps)
            if dve_subs is None:
                nc.vector.scalar_tensor_tensor(dst, tv, 0.01, tv,
                                               op0=OP.mult, op1=OP.max)
            else:
                for dst_i, c0, c1 in dve_subs:
                    nc.vector.scalar_tensor_tensor(dst_i, tv[:, c0:c1], 0.01,
                                                   tv[:, c0:c1],
                                                   op0=OP.mult, op1=OP.max)

    # ---- lin -> x1 V-windows ----
    psA = pp128.tile([128, BCORE], F32, tag="p128", name="lin0")
    nc.tensor.matmul(psA[:], W("lin0"), latT[:], start=True, stop=True)
    psB = pp128.tile([128, BCORE], F32, tag="p128", name="lin1")
    nc.tensor.matmul(psB[:], W("lin1"), latT[:], start=True, stop=True)
    for k in range(NV):
        # V_k rows = wi k..k+2 ; psA = wi 0..3, psB = wi 4..7
        for ps, base in ((psA, 0), (psB, 4)):
            lo = max(k, base)
            hi = min(k + 3, base + 4)
            if lo >= hi:
                continue
            p0, d0, n = (lo - base) * 32, (lo - k) * 32, (hi - lo) * 32
            for off, cnt in _pieces(p0, d0, n):
                lrelu_evac(x1t[k][d0 + off:d0 + off + cnt, :],
                           ps[p0 + off:p0 + off + cnt, :], cnt, BCORE)

    # ---- fused1 -> x3 (fill-owned j ranges) ----
    def x3_owner(j):
        for w, (lo, hi) in enumerate(X3_OWN):
            if lo <= j < hi:
                return w
        raise AssertionError(j)

    for ap_ in range(8):
        k = min(max(ap_ - 1, 0), NV - 1)
        psa = pp128.tile([128, BCORE], F32, tag="p128", name=f"f1a{ap_}")
        nc.tensor.matmul(psa[:], W(("f1a", ap_)), x1t[k][:], start=True, stop=True)
        psb = pp64.tile([64, BCORE], F32, tag="p64", name=f"f1b{ap_}")
        nc.tensor.matmul(psb[0:32, :], W(("f1b", ap_)), x1t[k][:], start=True, stop=True)
        j = 2 * ap_  # j-pair {j, j+1}, same owner by construction
        w = x3_owner(j)
        assert x3_owner(j + 1) == w, (j,)
        j0 = X3_WIN[w][0]
        d0 = (j - j0) * 16
        for hh in range(5):
            src = psa[hh * 32:(hh + 1) * 32, :] if hh < 4 else psb[0:32, :]
            p0 = hh * 32 if hh < 4 else 0
            assert (d0 - p0) % 32 == 0 or True
            fsl = slice(hh * BCORE, (hh + 1) * BCORE)
            lrelu_evac(x3t[w][d0:d0 + 32, fsl], src, 32, BCORE)

    # x3 mirrors: fill window overlaps from owners (SBUF-SBUF DMA)
    def x3_mirror(dst_w, j_lo, j_hi, src_w):
        dj0, sj0 = X3_WIN[dst_w][0], X3_WIN[src_w][0]
        nc.sync.dma_start(
            x3t[dst_w][(j_lo - dj0) * 16:(j_hi - dj0) * 16, :],
            x3t[src_w][(j_lo - sj0) * 16:(j_hi - sj0) * 16, :])

    x3_mirror(0, 2, 5, 1)    # w0 needs j 2..4 from w1
    x3_mirror(1, 8, 9, 2)    # w1 needs j 8
    x3_mirror(2, 6, 8, 1)    # w2 needs j 6..7
    x3_mirror(2, 12, 13, 3)  # w2 needs j 12
    x3_mirror(3, 10, 12, 2)  # w3 needs j 10..11

    # ---- fused2 -> a3 ----
    def xmv(a, i):
        K = X3_WIN[a][1] * 16
        return x3t[a][0:K + 1, i * BCORE:(i + 1) * BCORE]

    def f2_evac(ps, p_base, hh, w_lo, w_hi):
        """psum rows p_base+(W-w_lo)*8+c3 -> a3 owner rows, H col hh."""
        for t, (o_lo, o_hi) in enumerate(A3_OWN):
            lo = max(w_lo, o_lo)
            hi = min(w_hi, o_hi)
            if lo >= hi:
                continue
            W0 = A3_WIN[t][0]
            d0 = (lo - W0) * 8
            p0 = p_base + (lo - w_lo) * 8
            n = (hi - lo) * 8
            fsl = slice(hh * BCORE, (hh + 1) * BCORE)
            for off, cnt in _pieces(p0, d0, n):
                lrelu_evac(a3t[t][d0 + off:d0 + off + cnt, fsl],
                           ps[p0 + off:p0 + off + cnt, :], cnt, BCORE)

    for i in range(5):
        for a in range(4):
            ps = pp128.tile([128, BCORE], F32, tag="p128", name=f"p12_{i}_{a}")
            nc.tensor.matmul(ps[:], W(("r12", a)), xmv(a, i), start=True, stop=True)
            for ri, r in enumerate((1, 2)):
                f2_evac(ps, ri * 64, 5 * i + r, 8 * a, 8 * a + 8)
            ps = pp64.tile([64, BCORE], F32, tag="p64", name=f"p3_{i}_{a}")
            nc.tensor.matmul(ps[:], W(("r3", a)), xmv(a, i), start=True, stop=True)
            f2_evac(ps, 0, 5 * i + 3, 8 * a, 8 * a + 8)
            ps = pp64.tile([64, BCORE], F32, tag="p64", name=f"p0_{i}_{a}")
            nc.tensor.matmul(ps[:], W(("r0m", a, "edge" if i == 0 else "mid")),
                             xmv(a, i), start=True, stop=(i == 0))
            if i > 0:
                nc.tensor.matmul(ps[:], W(("r0h", a)), xmv(a, i - 1),
                                 start=False, stop=True)
            f2_evac(ps, 0, 5 * i, 8 * a, 8 * a + 8)
            ps = pp64.tile([64, BCORE], F32, tag="p64", name=f"p4_{i}_{a}")
            nc.tensor.matmul(ps[:], W(("r4m", a, "edge" if i == 4 else "mid")),
                             xmv(a, i), start=True, stop=(i == 4))
            if i < 4:
                nc.tensor.matmul(ps[:], W(("r4h", a)), xmv(a, i + 1),
                                 start=False, stop=True)
            f2_evac(ps, 0, 5 * i + 4, 8 * a, 8 * a + 8)

    # a3 mirrors: t0 rows 96..119 (W 12..14) <- t1 rows 0..23;
    #             t1 rows 64..119 (W 20..26) <- t2 rows 0..55
    nc.sync.dma_start(a3t[0][96:120, :], a3t[1][0:24, :])
    nc.sync.dma_start(a3t[1][64:120, :], a3t[2][0:56, :])

    ps_ctx.close()
    pA = ctx.enter_context(tc.tile_pool(name="pA", bufs=4, space="PSUM"))
    pB = ctx.enter_context(tc.tile_pool(name="pB", bufs=4, space="PSUM"))

    # ---- fused3 ----
    NT = [2 * 6 * (A3_JC[t][1] - A3_JC[t][0]) * 2 for t in range(3)]  # 312,312,144

    def st_ap(t, i, c):
        return a3t[t][0:a3K[t] + 1, i * BCORE + c * CH:i * BCORE + (c + 1) * CH]

    for c in range(4):
        slot = {}

        def slot_tiles(k):
            if k not in slot:
                ta = pA.tile([128, NT[0]], F32, tag="pA", name=f"sA_{c}_{k}")
                tb = pB.tile([128, NT[1] + NT[2]], F32, tag="pB", name=f"sB_{c}_{k}")
                slot[k] = (ta, tb)
            return slot[k]

        def regions(k):
            ta, tb = slot_tiles(k)
            return [ta[:, 0:NT[0]], tb[:, 0:NT[1]], tb[:, NT[1]:NT[1] + NT[2]]]

        stg = None

        def evac_slot(k):
            """slot k = rows {2k+1, 2k+2} clipped to [0,49]."""
            rlo = max(2 * k + 1, 0)
            rhi = min(2 * k + 2, 49)
            half = 0 if rlo >= 25 * 0 and rhi < 25 else 1
            hb = 25 * half
            stgw = stg[:].rearrange("p (c4 h jc s) -> p h c4 jc s",
                                    c4=6, h=25, jc=32, s=2)
            for t in range(3):
                lo, hi = A3_JC[t]
                reg = regions(k)[t]
                ht = NT[t] // 2
                if 2 * k + 1 < 0:       # slot -1: row 0 only (q=1 half)
                    src = reg[:, ht:NT[t]]
                    dst = stgw[:, 0:1, :, lo:hi, :]
                    subs = [(dst, 0, ht)]
                elif 2 * k + 2 > 49:    # slot 24: row 49 only (q=0 half)
                    src = reg[:, 0:ht]
                    dst = stgw[:, 49 - hb:50 - hb, :, lo:hi, :]
                    subs = [(dst, 0, ht)]
                else:
                    src = reg[:]
                    dst = stgw[:, rlo - hb:rhi + 1 - hb, :, lo:hi, :]
                    subs = [(stgw[:, rlo - hb:rlo - hb + 1, :, lo:hi, :], 0, ht),
                            (stgw[:, rhi - hb:rhi - hb + 1, :, lo:hi, :], ht, NT[t])]
                lrelu_evac(dst, src, 128, src.shape[1], dve_subs=subs)

        for i in range(25):
            if i == 0:
                stg = stgp.tile([128, 9600], BF16, tag="stg", name=f"stg_{c}_0")
            for t in range(3):
                st = st_ap(t, i, c)
                # PSUM start=True marks the whole 2KB bank pending-zero, so
                # only the FIRST write into each bank per slot-generation may
                # carry it; t=2 shares the pB bank with t=1 and must open with
                # start=False (its bytes are pending-zero -> fresh write).
                first_in_bank = t != 2
                # U: close slot i-1
                if i == 0:
                    reg = regions(-1)[t]
                    ht = NT[t] // 2
                    nc.tensor.matmul(reg[:, ht:NT[t]], st, W(("U0", t)),
                                     start=first_in_bank, stop=True,
                                     skip_group_check=True)
                else:
                    reg = regions(i - 1)[t]
                    nc.tensor.matmul(reg[:], st, W(("U", t)),
                                     start=False, stop=True, skip_group_check=True)
                # L: open slot i
                if i == 24:
                    reg = regions(24)[t]
                    ht = NT[t] // 2
                    nc.tensor.matmul(reg[:, 0:ht], st, W(("L24", t)),
                                     start=first_in_bank, stop=True,
                                     skip_group_check=True)
                else:
                    reg = regions(i)[t]
                    nc.tensor.matmul(reg[:], st, W(("L", t)),
                                     start=first_in_bank, stop=False,
                                     skip_group_check=True)
            evac_slot(i - 1)
            del slot[i - 1]
            if i - 1 == 11:  # rows 0..24 complete -> flush half 0
                sv = stg[:].rearrange("p (c4 h w) -> p c4 h w", c4=6, h=25, w=64)
                nc.sync.dma_start(out_ap[c * CH:(c + 1) * CH, :, 0:25, :], sv)
                stg = stgp.tile([128, 9600], BF16, tag="stg", name=f"stg_{c}_1")
        evac_slot(24)
        sv = stg[:].rearrange("p (c4 h w) -> p c4 h w", c4=6, h=25, w=64)
        nc.sync.dma_start(out_ap[c * CH:(c + 1) * CH, :, 25:50, :], sv)


def build_inmaps(inputs):
    import ml_dtypes
    wp = _make_packs(inputs)
    wpack = wp.build().astype(ml_dtypes.bfloat16)
    _build_program.wreg = wp.reg
    lat = np.asarray(inputs["latent"], np.float32)
    ones = np.ones((1, 12800), ml_dtypes.bfloat16)
    in_maps = []
    for i in range(NCORES):
        latT = np.ones((8, BCORE), np.float32)
        latT[:7] = lat[i * BCORE:(i + 1) * BCORE].T
        in_maps.append({"latT": latT.astype(ml_dtypes.bfloat16), "wpack": wpack,
                        "ones": ones})
    return in_maps, wpack.shape[1]


def kernel(**inputs):
    inputs = {k: np.asarray(v) for k, v in inputs.items()}
    in_maps, wcols = build_inmaps(inputs)
    nc = _build_program(wcols)
    res = run_bass_kernel_spmd(nc, in_maps, core_ids=list(range(NCORES)))
    out = np.concatenate([np.asarray(res.results[i]["out"]) for i in range(NCORES)],
                         axis=0)
    return out.astype(np.float32)


# revision 7
# speedup vs baseline: 2.5748x; 1.5469x over previous
"""Trainium2 Bass kernel for nn_BetaVAEMark7Decoder (v3).

All six layers are banded matmuls on the TensorEngine in bf16, data-parallel
over batch (4096 rows -> 512 per NeuronCore).  Biases ride as extra rows of
the stationary operands (activation tiles carry a constant ones-row; the f1
stage uses K=1 ones-stationary bias matmuls), so every PSUM evacuation is a
single bias-free leaky-relu instruction spread across the Scalar and Vector
engines.  fused2 evacuates full [128,512] tiles into x4 staging tiles; the
fused3 input windows are then built with SBUF-SBUF DMAs (idle DMA engines)
instead of fragmented partition-piece copies.  The final layer is blocked on
odd output-row boundaries (slot k = rows {2k+1,2k+2}) so each input slice
feeds exactly two PSUM slots.  Output is staged batch-major in bf16 as
(h, c4, w) and written with 8 large contiguous DMAs; the host transposes to
NCHW and upcasts to float32.
"""
import numpy as np
from contextlib import ExitStack

import concourse.bass as bass
import concourse.tile as tile
from concourse import bacc, mybir
from concourse.bass_utils import run_bass_kernel_spmd

F32 = mybir.dt.float32
BF16 = mybir.dt.bfloat16
AF = mybir.ActivationFunctionType
OP = mybir.AluOpType

NCORES = 8
BCORE = 512
CH = 128

# x1 windows: W_a holds wi in [w0, w0+4)
X1W = [(0, 4), (1, 4), (3, 4), (4, 4)]
# x3 (= post-fused1, j in 0..15, c2 in 0..15, split c2h halves) j-windows
X3_WIN = [(0, 9), (4, 12)]      # (j0, nj): A = j 0..8, B = j 4..15
# a3 (= x4, W in 0..31, c3 in 0..7) windows for fused3: (W0, nW)
A3_WIN = [(0, 15), (12, 15), (20, 12)]
# fused3 weight-col ownership (jc ranges) per window
A3_JC = [(0, 13), (13, 26), (26, 32)]


# ---------------- host-side weight factorization ----------------
def _precompute(w):
    P = {}
    w_lin, b_lin = w["w_lin"], w["b_lin"]
    lhs_lin = np.zeros((7, 256), np.float32)
    c_lin = np.zeros(256, np.float32)
    for wi in range(8):
        for ci in range(32):
            lhs_lin[:, wi * 32 + ci] = w_lin[:, ci * 8 + wi]
            c_lin[wi * 32 + ci] = b_lin[ci * 8 + wi]
    P["lhs_lin"], P["c_lin"] = lhs_lin, c_lin

    w_up1, b_up1, w_tc1, b_tc1 = w["w_up1"], w["b_up1"], w["w_tc1"], w["b_tc1"]
    K1 = np.zeros((5, 2, 3, 32, 16), np.float32)
    for hh in range(5):
        for s in range(2):
            for dh in range(3):
                hp = hh + 1 - dh
                if not (0 <= hp < 5):
                    continue
                for dw in range(3):
                    t = s + 1 - dw
                    dj = int(np.floor(t / 2))
                    kw = t - 2 * dj
                    K1[hh, s, dj + 1] += np.einsum("ic,cd->id", w_up1[hp, kw], w_tc1[dh, dw])
    c1 = np.zeros((5, 16, 16), np.float32)
    for hh in range(5):
        for ww in range(16):
            acc = b_tc1.copy()
            for dh in range(3):
                if not (0 <= hh + 1 - dh < 5):
                    continue
                for dw in range(3):
                    if not (0 <= ww + 1 - dw < 16):
                        continue
                    acc = acc + b_up1 @ w_tc1[dh, dw]
            c1[hh, ww] = acc
    P["K1"], P["c1"] = K1, c1

    w_up2, b_up2, w_tc2, b_tc2 = w["w_up2"], w["b_up2"], w["w_tc2"], w["b_tc2"]
    K2 = np.zeros((5, 2, 3, 3, 16, 8), np.float32)
    for r in range(5):
        for s in range(2):
            for dh in range(3):
                u = r + 1 - dh
                di = int(np.floor(u / 5))
                kh = u - 5 * di
                for dw in range(3):
                    t = s + 1 - dw
                    dj = int(np.floor(t / 2))
                    kw = t - 2 * dj
                    K2[r, s, di + 1, dj + 1] += np.einsum("ic,cd->id", w_up2[kh, kw], w_tc2[dh, dw])
    P["K2"] = K2
    P["BB2"] = np.einsum("c,hwcd->hwd", b_up2, w_tc2)
    P["b_tc2"] = b_tc2

    w_up3, b_up3, w_tc3, b_tc3 = w["w_up3"], w["b_up3"], w["w_tc3"], w["b_tc3"]
    K3 = np.zeros((2, 2, 3, 3, 8, 6), np.float32)
    for r in range(2):
        for s in range(2):
            for dh in range(3):
                u = r + 1 - dh
                di = int(np.floor(u / 2))
                kh = u - 2 * di
                for dw in range(3):
                    t = s + 1 - dw
                    dj = int(np.floor(t / 2))
                    kw = t - 2 * dj
                    K3[r, s, di + 1, dj + 1] += np.einsum("ic,cd->id", w_up3[kh, kw], w_tc3[dh, dw])
    P["K3"] = K3
    P["BB3"] = np.einsum("c,hwcd->hwd", b_up3, w_tc3)
    P["b_tc3"] = b_tc3
    return P


def _fused1_blocks(P):
    """Per (a = x3-j quad, hg = H group {0,1},{2,3},{4}): weight block
    [128, M] (rows = X1 window wi*32+ci) and bias block [1, M].
    Cols = (hi, c2h, wl, c2l)."""
    K1, c1 = P["K1"], P["c1"]
    blocks = {}
    for a in range(4):
        w0 = X1W[a][0]
        for hg in range(3):
            nh = 2 if hg < 2 else 1
            M = nh * 64
            B = np.zeros((128, M), np.float32)
            bias = np.zeros((1, M), np.float32)
            for hi in range(nh):
                hh = hg * 2 + hi
                for c2h in range(2):
                    for wl in range(4):
                        j = 4 * a + wl
                        ju, s = j // 2, j % 2
                        for c2l in range(8):
                            c2 = c2h * 8 + c2l
                            col = hi * 64 + c2h * 32 + wl * 8 + c2l
                            bias[0, col] = c1[hh, j, c2]
                            for wi_l in range(4):
                                wi = w0 + wi_l
                                dj = wi - ju
                                if -1 <= dj <= 1:
                                    B[wi_l * 32:(wi_l + 1) * 32, col] = K1[hh, s, dj + 1, :, c2]
            blocks[("f1w", a, hg)] = B
            blocks[("f1b", a, hg)] = bias
    return blocks


def _fused2_blocks(P):
    """Blocks per (half, r, c2h [, variants]): [K+1, 128] with rows =
    x3-window (j, c2l) and ones/bias row at K.  Cols = (wl 16, c3 8).
    c2h=0 main blocks carry the bias row; c2h=1 and halo blocks are zero."""
    K2, BB2, b_tc2 = P["K2"], P["BB2"], P["b_tc2"]

    def col_bias(Hh, Ww, c3):
        acc = b_tc2[c3]
        for dh in range(3):
            if not (0 <= Hh + 1 - dh < 25):
                continue
            for dw in range(3):
                if not (0 <= Ww + 1 - dw < 32):
                    continue
                acc += BB2[dh, dw, c3]
        return acc

    blocks = {}
    for half in range(2):
        win = 0 if half == 0 else 1
        j0, nj = X3_WIN[win]
        Kr = nj * 8

        def base(r, di, c2h):
            B = np.zeros((Kr + 1, 128), np.float32)
            for wl in range(16):
                Ww = 16 * half + wl
                j, s = Ww // 2, Ww % 2
                for c3 in range(8):
                    col = wl * 8 + c3
                    for jl in range(nj):
                        dj = (j0 + jl) - j
                        if -1 <= dj <= 1:
                            B[jl * 8:(jl + 1) * 8, col] = \
                                K2[r, s, di + 1, dj + 1, c2h * 8:(c2h + 1) * 8, c3]
            return B

        def add_bias(B, Hh):
            for wl in range(16):
                for c3 in range(8):
                    B[Kr, wl * 8 + c3] = col_bias(Hh, 16 * half + wl, c3)
            return B

        for r in range(5):
            for c2h in range(2):
                B = base(r, 0, c2h)
                if c2h == 1:
                    blocks[("f2", half, r, 1, "m")] = B
                    continue
                if r in (1, 2, 3):
                    blocks[("f2", half, r, 0, "m")] = add_bias(B.copy(), 5 + r)
                else:
                    edge_i = 0 if r == 0 else 4
                    blocks[("f2", half, r, 0, "mid")] = add_bias(B.copy(), 10 + r)
                    blocks[("f2", half, r, 0, "edge")] = add_bias(B.copy(), 5 * edge_i + r)
        for c2h in range(2):
            blocks[("f2", half, 0, c2h, "h")] = base(0, -1, c2h)
            blocks[("f2", half, 4, c2h, "h")] = base(4, 1, c2h)
    return blocks


def _fused3_blocks(P):
    """Slot-prime blocks.  Per window t: rows (W-W0)*8+c3, ones row at 8*nW.
    Cols ordered (q, c4, jc-own, s).  U: q=0 -> row 2i-1 (di=+1), q=1 -> row 2i
    (di=0, carries bias).  L: q=0 -> row 2i+1 (di=0, bias), q=1 -> row 2i+2
    (di=-1).  U0 = q=1 half with H'=0 edge bias; L24 = q=0 half, H'=49 edge."""
    K3, BB3, b_tc3 = P["K3"], P["BB3"], P["b_tc3"]

    def col_bias(jc, s, c4, drop_dh):
        acc = b_tc3[c4]
        for dh in range(3):
            if dh in drop_dh:
                continue
            for dw in range(3):
                tt = s + 1 - dw
                dj = int(np.floor(tt / 2))
                if 0 <= jc + dj < 32:
                    acc += BB3[dh, dw, c4]
        return acc

    blocks = {}
    for t in range(3):
        W0, nW = A3_WIN[t]
        lo, hi = A3_JC[t]
        njc = hi - lo
        K = 8 * nW
        half = 6 * njc * 2

        def fill(B, q, r, di, bias_drop=None):
            for c4 in range(6):
                for jci in range(njc):
                    jc = lo + jci
                    for s in range(2):
                        col = q * half + (c4 * njc + jci) * 2 + s
                        if bias_drop is not None:
                            B[K, col] = col_bias(jc, s, c4, bias_drop)
                        for dj in (-1, 0, 1):
                            j = jc + dj
                            if not (W0 <= j < W0 + nW) or not (0 <= j < 32):
                                continue
                            jl = j - W0
                            B[jl * 8:(jl + 1) * 8, col] = K3[r, s, di + 1, dj + 1, :, c4]

        U = np.zeros((K + 1, 2 * half), np.float32)
        fill(U, 0, 1, +1)                      # row 2i-1
        fill(U, 1, 0, 0, bias_drop=set())      # row 2i, interior bias
        blocks[("U", t)] = U
        L = np.zeros((K + 1, 2 * half), np.float32)
        fill(L, 0, 1, 0, bias_drop=set())      # row 2i+1, interior bias
        fill(L, 1, 0, -1)                      # row 2i+2
        blocks[("L", t)] = L
        U0 = np.zeros((K + 1, 2 * half), np.float32)
        fill(U0, 1, 0, 0, bias_drop={2})       # row 0 edge bias
        blocks[("U0", t)] = U0[:, half:]
        L24 = np.zeros((K + 1, 2 * half), np.float32)
        fill(L24, 0, 1, 0, bias_drop={0})      # row 49 edge bias
        blocks[("L24", t)] = L24[:, :half]
    return blocks


class _Pack:
    """Packs [K, M] blocks into one [128, cols] array; remembers offsets."""

    def __init__(self):
        self.cols = 0
        self.reg = {}
        self.items = []

    def add(self, key, arr):
        K, M = arr.shape
        self.reg[key] = (self.cols, K, M)
        self.items.append(arr)
        self.cols += M

    def build(self):
        out = np.zeros((128, self.cols), np.float32)
        c = 0
        for arr in self.items:
            K, M = arr.shape
            out[:K, c:c + M] = arr
            c += M
        return out


def _make_packs(inputs):
    P = _precompute(inputs)
    f1b = _fused1_blocks(P)
    f2b = _fused2_blocks(P)
    f3b = _fused3_blocks(P)

    wp = _Pack()
    lin0 = np.zeros((8, 128), np.float32)
    lin0[:7] = P["lhs_lin"][:, 0:128]
    lin0[7] = P["c_lin"][0:128]
    lin1 = np.zeros((8, 128), np.float32)
    lin1[:7] = P["lhs_lin"][:, 128:256]
    lin1[7] = P["c_lin"][128:256]
    wp.add("lin0", lin0)
    wp.add("lin1", lin1)
    for a in range(4):
        for hg in range(3):
            wp.add(("f1w", a, hg), f1b[("f1w", a, hg)])
            wp.add(("f1b", a, hg), f1b[("f1b", a, hg)])
    nearly = wp.cols
    for key, arr in f2b.items():
        wp.add(key, arr)
    for t in range(3):
        for kind in ("U", "L", "U0", "L24"):
            wp.add((kind, t), f3b[(kind, t)])
    return wp, nearly


# ---------------- device program ----------------
_PROG = {}


def _lim(s):
    s = s % 128
    if s == 0:
        return 128
    if s == 64:
        return 64
    return 32


def _pieces(p0, d0, n):
    assert p0 % 32 == 0 and d0 % 32 == 0, (p0, d0, n)
    out = []
    off = 0
    while off < n:
        s1, s2 = (p0 + off) % 128, (d0 + off) % 128
        c = min(n - off, _lim(s1), _lim(s2))
        out.append((off, c))
        off += c
    return out


def _build_program(wcols, nearly):
    key = (wcols, nearly)
    if key in _PROG:
        return _PROG[key]
    nc = bacc.Bacc("TRN2", target_bir_lowering=False, debug=False, num_devices=NCORES)
    lat_ap = nc.dram_tensor("latT", [8, BCORE], BF16, kind="ExternalInput").ap()
    wp_ap = nc.dram_tensor("wpack", [128, wcols], BF16, kind="ExternalInput").ap()
    ones_ap = nc.dram_tensor("ones", [1, 12800], BF16, kind="ExternalInput").ap()
    out_ap = nc.dram_tensor("out", [BCORE, 50, 6, 64], BF16, kind="ExternalOutput").ap()
    with tile.TileContext(nc) as tc:
        with ExitStack() as ctx:
            _emit(ctx, tc, nc, lat_ap, wp_ap, ones_ap, out_ap,
                  _build_program.wreg, nearly)
    nc.compile()
    _PROG[key] = nc
    return nc


def _emit(ctx, tc, nc, lat_ap, wp_ap, ones_ap, out_ap, wreg, nearly):
    wcols = wp_ap.shape[1]

    consts = ctx.enter_context(tc.tile_pool(name="consts", bufs=1))
    x1p = ctx.enter_context(tc.tile_pool(name="x1", bufs=1))
    x3p = ctx.enter_context(tc.tile_pool(name="x3", bufs=1))
    x4p0 = ctx.enter_context(tc.tile_pool(name="x4a", bufs=2))
    x4p1 = ctx.enter_context(tc.tile_pool(name="x4b", bufs=2))
    a3p = ctx.enter_context(tc.tile_pool(name="a3", bufs=1))
    stgp = ctx.enter_context(tc.tile_pool(name="stg", bufs=2))
    tmpp = ctx.enter_context(tc.tile_pool(name="tmp", bufs=4))
    ps_ctx = ExitStack()
    pp128 = ps_ctx.enter_context(tc.tile_pool(name="pp128", bufs=6, space="PSUM"))

    # ---- constants: split weight DMA so lin/f1 start early ----
    wbt = consts.tile([128, wcols], BF16)
    nc.sync.dma_start(wbt[:, 0:nearly], wp_ap[:, 0:nearly])
    nc.sync.dma_start(wbt[:, nearly:wcols], wp_ap[:, nearly:wcols])
    latT = consts.tile([8, BCORE], BF16)
    nc.sync.dma_start(latT[:], lat_ap[:])
    onesr = consts.tile([1, BCORE], BF16)
    nc.sync.dma_start(onesr[:], ones_ap[:, 0:BCORE])

    def W(key):
        o, K, M = wreg[key]
        return wbt[:K, o:o + M]

    # activation tiles
    x1t = [x1p.tile([128, BCORE], BF16, name=f"x1_{a}") for a in range(4)]
    # x3 tiles keyed (win, c2h); ones row at nj*8
    x3t = {}
    for win in range(2):
        for c2h in range(2):
            Kr = X3_WIN[win][1] * 8
            t_ = x3p.tile([Kr + 1, 5 * BCORE], BF16, name=f"x3_{win}_{c2h}")
            x3t[(win, c2h)] = t_
            nc.sync.dma_start(t_[Kr:Kr + 1, :], ones_ap[:, 0:5 * BCORE])
    a3K = [A3_WIN[t][1] * 8 for t in range(3)]
    a3t = [a3p.tile([a3K[t] + 1, 25 * BCORE], BF16, name=f"a3_{t}") for t in range(3)]
    for t in range(3):
        nc.sync.dma_start(a3t[t][a3K[t]:a3K[t] + 1, :], ones_ap[:, 0:25 * BCORE])

    # ---- evac dispatcher: lrelu(psum) -> dst ----
    ev_n = [0]

    def lrelu_evac(dst, ps, np_, nf):
        k = ev_n[0]
        ev_n[0] += 1
        m = k % 8
        if m < 5:
            nc.scalar.activation(dst, ps, AF.Lrelu, bias=0.0, scale=1.0, alpha=0.01)
        else:
            tmp = tmpp.tile([128, 512], BF16, tag="evt", name=f"evt{k}")
            tv = tmp[0:np_, 0:nf]
            nc.vector.tensor_copy(tv, ps)
            nc.vector.scalar_tensor_tensor(dst, tv, 0.01, tv, op0=OP.mult, op1=OP.max)

    # ---- lin -> x1 windows ----
    psA = pp128.tile([128, BCORE], F32, tag="p128", name="lin0")
    nc.tensor.matmul(psA[:], W("lin0"), latT[:], start=True, stop=True)
    psB = pp128.tile([128, BCORE], F32, tag="p128", name="lin1")
    nc.tensor.matmul(psB[:], W("lin1"), latT[:], start=True, stop=True)
    for a in range(4):
        w0 = X1W[a][0]
        for ps, base in ((psA, 0), (psB, 4)):
            lo = max(w0, base)
            hi = min(w0 + 4, base + 4)
            if lo >= hi:
                continue
            p0, d0, n = (lo - base) * 32, (lo - w0) * 32, (hi - lo) * 32
            for off, cnt in _pieces(p0, d0, n):
                lrelu_evac(x1t[a][d0 + off:d0 + off + cnt, :],
                           ps[p0 + off:p0 + off + cnt, :], cnt, BCORE)

    # ---- fused1 -> x3 (c2h-halved windows) ----
    # fill ownership: window A owns j 0..3, B owns j 4..15
    for a in range(4):
        for hg in range(3):
            nh = 2 if hg < 2 else 1
            M = nh * 64
            ps = pp128.tile([128, BCORE], F32, tag="p128", name=f"f1_{a}_{hg}")
            nc.tensor.matmul(ps[0:M, :], W(("f1w", a, hg)), x1t[a][:],
                             start=True, stop=False)
            nc.tensor.matmul(ps[0:M, :], W(("f1b", a, hg)), onesr[:],
                             start=False, stop=True)
            win = 0 if a == 0 else 1
            j0 = X3_WIN[win][0]
            d0 = (4 * a - j0) * 8
            for hi in range(nh):
                hh = hg * 2 + hi
                fsl = slice(hh * BCORE, (hh + 1) * BCORE)
                for c2h in range(2):
                    p0 = hi * 64 + c2h * 32
                    lrelu_evac(x3t[(win, c2h)][d0:d0 + 32, fsl],
                               ps[p0:p0 + 32, :], 32, BCORE)

    # x3 mirror: window A rows j 4..8 <- window B rows 0..40
    for c2h in range(2):
        nc.sync.dma_start(x3t[(0, c2h)][32:72, :], x3t[(1, c2h)][0:40, :])

    # ---- fused2 -> x4 staging -> a3 windows via DMA ----
    def xmv(half, c2h, i):
        win = 0 if half == 0 else 1
        Kr = X3_WIN[win][1] * 8
        return x3t[(win, c2h)][0:Kr + 1, i * BCORE:(i + 1) * BCORE]

    for i in range(5):
        x4 = [x4p0.tile([128, 5 * BCORE], BF16, tag="x4a", name=f"x4_{i}_0"),
              x4p1.tile([128, 5 * BCORE], BF16, tag="x4b", name=f"x4_{i}_1")]
        for half in range(2):
            for r in range(5):
                ps = pp128.tile([128, BCORE], F32, tag="p128", name=f"f2_{i}_{half}_{r}")
                if r in (1, 2, 3):
                    tag0 = "m"
                else:
                    edge_i = 0 if r == 0 else 4
                    tag0 = "edge" if i == edge_i else "mid"
                nc.tensor.matmul(ps[:], W(("f2", half, r, 0, tag0)),
                                 xmv(half, 0, i), start=True, stop=False)
                last = r in (1, 2, 3) or i == (0 if r == 0 else 4)
                nc.tensor.matmul(ps[:], W(("f2", half, r, 1, "m")),
                                 xmv(half, 1, i), start=False, stop=last)
                if r == 0 and i > 0:
                    nc.tensor.matmul(ps[:], W(("f2", half, 0, 0, "h")),
                                     xmv(half, 0, i - 1), start=False, stop=False)
                    nc.tensor.matmul(ps[:], W(("f2", half, 0, 1, "h")),
                                     xmv(half, 1, i - 1), start=False, stop=True)
                if r == 4 and i < 4:
                    nc.tensor.matmul(ps[:], W(("f2", half, 4, 0, "h")),
                                     xmv(half, 0, i + 1), start=False, stop=False)
                    nc.tensor.matmul(ps[:], W(("f2", half, 4, 1, "h")),
                                     xmv(half, 1, i + 1), start=False, stop=True)
                lrelu_evac(x4[half][:, r * BCORE:(r + 1) * BCORE], ps[:], 128, BCORE)
        # build a3 window slices for H rows 5i..5i+4 (cols i*2560..)
        csl = slice(i * 5 * BCORE, (i + 1) * 5 * BCORE)
        nc.sync.dma_start(a3t[0][0:120, csl], x4[0][0:120, :])
        nc.sync.dma_start(a3t[1][0:32, csl], x4[0][96:128, :])
        nc.sync.dma_start(a3t[1][32:120, csl], x4[1][0:88, :])
        nc.sync.dma_start(a3t[2][0:96, csl], x4[1][32:128, :])

    ps_ctx.close()
    pA = ctx.enter_context(tc.tile_pool(name="pA", bufs=4, space="PSUM"))
    pB = ctx.enter_context(tc.tile_pool(name="pB", bufs=4, space="PSUM"))

    # ---- fused3 ----
    NT = [2 * 6 * (A3_JC[t][1] - A3_JC[t][0]) * 2 for t in range(3)]  # 312,312,144

    def st_ap(t, i, c):
        return a3t[t][0:a3K[t] + 1, i * BCORE + c * CH:i * BCORE + (c + 1) * CH]

    for c in range(4):
        slot = {}

        def slot_tiles(k):
            if k not in slot:
                ta = pA.tile([128, NT[0]], F32, tag="pA", name=f"sA_{c}_{k}")
                tb = pB.tile([128, NT[1] + NT[2]], F32, tag="pB", name=f"sB_{c}_{k}")
                slot[k] = (ta, tb)
            return slot[k]

        def regions(k):
            ta, tb = slot_tiles(k)
            return [ta[:, 0:NT[0]], tb[:, 0:NT[1]], tb[:, NT[1]:NT[1] + NT[2]]]

        stg = None

        def evac_slot(k):
            rlo = max(2 * k + 1, 0)
            rhi = min(2 * k + 2, 49)
            half = 0 if rhi < 25 else 1
            hb = 25 * half
            stgw = stg[:].rearrange("p (h c4 jc s) -> p h c4 jc s",
                                    h=25, c4=6, jc=32, s=2)
            for t in range(3):
                lo, hi = A3_JC[t]
                reg = regions(k)[t]
                ht = NT[t] // 2
                if 2 * k + 1 < 0:       # slot -1: row 0 only (q=1 half)
                    src = reg[:, ht:NT[t]]
                    dst = stgw[:, 0:1, :, lo:hi, :]
                elif 2 * k + 2 > 49:    # slot 24: row 49 only (q=0 half)
                    src = reg[:, 0:ht]
                    dst = stgw[:, 49 - hb:50 - hb, :, lo:hi, :]
                else:
                    src = reg[:]
                    dst = stgw[:, rlo - hb:rhi + 1 - hb, :, lo:hi, :]
                lrelu_evac(dst, src, 128, src.shape[1])

        for i in range(25):
            if i == 0:
                stg = stgp.tile([128, 9600], BF16, tag="stg", name=f"stg_{c}_0")
            for t in range(3):
                st = st_ap(t, i, c)
                # PSUM start=True marks the whole 2KB bank pending-zero, so
                # only the first write into each bank per slot-generation may
                # carry it; t=2 shares the pB bank with t=1.
                first_in_bank = t != 2
                if i == 0:
                    reg = regions(-1)[t]
                    ht = NT[t] // 2
                    nc.tensor.matmul(reg[:, ht:NT[t]], st, W(("U0", t)),
                                     start=first_in_bank, stop=True,
                                     skip_group_check=True)
                else:
                    reg = regions(i - 1)[t]
                    nc.tensor.matmul(reg[:], st, W(("U", t)),
                                     start=False, stop=True, skip_group_check=True)
                if i == 24:
                    reg = regions(24)[t]
                    ht = NT[t] // 2
                    nc.tensor.matmul(reg[:, 0:ht], st, W(("L24", t)),
                                     start=first_in_bank, stop=True,
                                     skip_group_check=True)
                else:
                    reg = regions(i)[t]
                    nc.tensor.matmul(reg[:], st, W(("L", t)),
                                     start=first_in_bank, stop=False,
                                     skip_group_check=True)
            evac_slot(i - 1)
            del slot[i - 1]
            if i - 1 == 11:  # rows 0..24 complete -> flush half 0
                sv = stg[:].rearrange("p (h c4 w) -> p h c4 w", h=25, c4=6, w=64)
                nc.sync.dma_start(out_ap[c * CH:(c + 1) * CH, 0:25, :, :], sv)
                stg = stgp.tile([128, 9600], BF16, tag="stg", name=f"stg_{c}_1")
        evac_slot(24)
        sv = stg[:].rearrange("p (h c4 w) -> p h c4 w", h=25, c4=6, w=64)
        nc.sync.dma_start(out_ap[c * CH:(c + 1) * CH, 25:50, :, :], sv)


def build_inmaps(inputs):
    import ml_dtypes
    wp, nearly = _make_packs(inputs)
    wpack = wp.build().astype(ml_dtypes.bfloat16)
    _build_program.wreg = wp.reg
    lat = np.asarray(inputs["latent"], np.float32)
    ones = np.ones((1, 12800), ml_dtypes.bfloat16)
    in_maps = []
    for i in range(NCORES):
        latT = np.ones((8, BCORE), np.float32)
        latT[:7] = lat[i * BCORE:(i + 1) * BCORE].T
        in_maps.append({"latT": latT.astype(ml_dtypes.bfloat16), "wpack": wpack,
                        "ones": ones})
    return in_maps, wpack.shape[1], nearly


def kernel(**inputs):
    inputs = {k: np.asarray(v) for k, v in inputs.items()}
    in_maps, wcols, nearly = build_inmaps(inputs)
    nc = _build_program(wcols, nearly)
    res = run_bass_kernel_spmd(nc, in_maps, core_ids=list(range(NCORES)))
    out = np.concatenate([np.asarray(res.results[i]["out"]) for i in range(NCORES)],
                         axis=0)
    return out.transpose(0, 2, 1, 3).astype(np.float32)
